# revision 1
# baseline (speedup 1.0000x reference)
"""Trainium2 Bass kernel for nn_MixtureLayer (MoE routing, 8 experts, top-2,
grouped capacity routing + shared expert).

Strategy: data-parallel over the 128 token-groups -> 16 groups per core.
Each core runs the router, dispatch, all 8 experts' FFNs on its own groups,
the shared expert, and the combine.  No collectives needed.

Numerics: router (logits/softmax/top-k/cumsum) entirely in fp32 so expert
selection matches the jax reference; the heavy FFN matmuls run in bf16 with
fp32 PSUM accumulation.
"""

import sys
import types

import numpy as np

try:  # concourse is normally on sys.path via the container's site setup
    import concourse.bass as bass  # noqa: F401
except ImportError:  # pragma: no cover
    sys.path.insert(0, "/opt/trn_rl_repo")

import concourse.bass as bass
import concourse.tile as tile
from concourse import bacc, mybir
from concourse.bass_utils import run_bass_kernel_spmd

F32 = mybir.dt.float32
BF16 = mybir.dt.bfloat16
AF = mybir.ActivationFunctionType
ALU = mybir.AluOpType
GELU = AF.Gelu_apprx_tanh  # jax.nn.gelu(approximate=True)

# ---- problem constants (hardcoded from the spec) ----
NCORES = 8
D, H, E = 1024, 4096, 8
B, S = 8, 2048
GRP = 128                 # tokens per routing group
NG_TOT = 128              # total groups
NG = NG_TOT // NCORES     # groups per core = 16
TOK = NG * GRP            # tokens per core = 2048
CAP = 32                  # capacity slots per (group, expert); slot 31 unused
DC = D // 128             # 8 chunks of d
HC = H // 128             # 32 chunks of h
SLOTS = NG * CAP          # 512 slots per expert per core

_CACHE = {}


def _ensure_ntff_hook():
    """Register the axon NTFF profiling hook if the image's antenv stub lacks
    it (needed only when tracing; harmless otherwise)."""
    try:
        import antenv
    except ImportError:
        return
    if "antenv.axon_hooks" in sys.modules:
        return
    m = types.ModuleType("antenv.axon_hooks")
    m._hook = None

    def _set(h, _m=m):
        _m._hook = h

    def _get(_m=m):
        return _m._hook

    m.set_axon_ntff_profile_hook = _set
    m.get_axon_ntff_profile_hook = _get
    sys.modules["antenv.axon_hooks"] = m
    antenv.axon_hooks = m
    try:
        from trn_agent_boot.trn_boot import _ntff_profile_via_ctypes

        hook = _ntff_profile_via_ctypes("/opt/axon/libaxon_pjrt.so")
        if hook is not None:
            _set(hook)
    except Exception:
        pass


def _emit_ffn_unit(nc, pools, rhs_fn, out_ap_fn,
                   keys_f32=None, keys_b16=None, vals_f32=None, vals_b16=None):
    """One FFN 'unit': 512 input columns (slots/tokens) through d->h gelu h->d.

    rhs_fn(dc) -> AP [128, 512] of the input in transposed layout (d on
    partitions).  Output rows [out_row0, out_row0+512) of out_dram (bf16).
    Weights stream from fp32 HBM (with inline bf16 cast) or from a
    pre-cast bf16 DRAM scratch.
    """
    hid = [pools["hid"].tile([128, 512], BF16, tag=f"hid{hc}", name=f"hid{hc}") for hc in range(HC)]
    # FFN1: hid[hc][128, 512] = gelu(sum_dc keys[dc,hc].T @ rhs[dc])
    for hcb in range(8):  # blocks of 4 h-chunks
        eps = [pools["ps"].tile([128, 512], F32, tag="ps", name="ps") for _ in range(4)]
        for dc in range(DC):
            if keys_f32 is not None:
                kf = pools["kf"].tile([128, 512], F32, tag="kf", name="kf")
                nc.sync.dma_start(kf[:], keys_f32[dc * 128:(dc + 1) * 128,
                                                 hcb * 512:(hcb + 1) * 512])
                kb = pools["kb"].tile([128, 512], BF16, tag="kb", name="kb")
                nc.vector.tensor_copy(kb[:], kf[:])
            else:
                kb = pools["kb"].tile([128, 512], BF16, tag="kb", name="kb")
                nc.sync.dma_start(kb[:], keys_b16[dc][:,
                                                 hcb * 512:(hcb + 1) * 512])
            rhs = rhs_fn(dc)
            for hh in range(4):
                nc.tensor.matmul(eps[hh][:], kb[:, hh * 128:(hh + 1) * 128], rhs,
                                 start=(dc == 0), stop=(dc == DC - 1))
        for hh in range(4):
            nc.scalar.activation(hid[hcb * 4 + hh][:], eps[hh][:], GELU)
    # FFN2: out[sc*128.., 1024] = sum_hc hid[hc][:,sc].T @ values[hc]
    pss = [[pools["ps"].tile([128, 512], F32, tag="ps", name="ps") for _ in range(2)]
           for _ in range(4)]
    for hc in range(HC):
        if vals_f32 is not None:
            vf = pools["vf"].tile([128, 1024], F32, tag="vf", name="vf")
            nc.sync.dma_start(vf[:], vals_f32[hc * 128:(hc + 1) * 128, :])
            vb = pools["vb"].tile([128, 1024], BF16, tag="vb", name="vb")
            nc.vector.tensor_copy(vb[:], vf[:])
        else:
            vb = pools["vb"].tile([128, 1024], BF16, tag="vb", name="vb")
            nc.sync.dma_start(vb[:], vals_b16[hc // 4][(hc % 4) * 128:
                                                      (hc % 4 + 1) * 128, :])
        for sc in range(4):
            lhsT = hid[hc][:, sc * 128:(sc + 1) * 128]
            nc.tensor.matmul(pss[sc][0][:], lhsT, vb[:, 0:512],
                             start=(hc == 0), stop=(hc == HC - 1))
            nc.tensor.matmul(pss[sc][1][:], lhsT, vb[:, 512:1024],
                             start=(hc == 0), stop=(hc == HC - 1))
    for sc in range(4):
        eo = pools["eo"].tile([128, 1024], BF16, tag="eo", name="eo")
        nc.scalar.copy(eo[:, 0:512], pss[sc][0][:])
        nc.scalar.copy(eo[:, 512:1024], pss[sc][1][:])
        nc.gpsimd.dma_start(out_ap_fn(sc), eo[:])


def _build_program():
    nc = bacc.Bacc("TRN2", target_bir_lowering=False, debug=False,
                   num_devices=NCORES)

    x_d = nc.dram_tensor("x_s", [TOK, D], F32, kind="ExternalInput").ap()
    gw_d = nc.dram_tensor("gw", [D, E], F32, kind="ExternalInput").ap()
    gb_d = nc.dram_tensor("gb", [1, E], F32, kind="ExternalInput").ap()
    k_d = nc.dram_tensor("keys", [E, D, H], F32, kind="ExternalInput").ap()
    v_d = nc.dram_tensor("values", [E, H, D], F32, kind="ExternalInput").ap()
    sk_d = nc.dram_tensor("shk", [D, H], F32, kind="ExternalInput").ap()
    sv_d = nc.dram_tensor("shv", [H, D], F32, kind="ExternalInput").ap()
    out_d = nc.dram_tensor("out", [TOK, D], F32, kind="ExternalOutput").ap()

    from contextlib import ExitStack
    with tile.TileContext(nc) as tc, ExitStack() as es_glob:
        # pool releases must be LIFO; phases close explicitly in stack order
        es_xtb, es_dT = ExitStack(), ExitStack()
        es_pre, es_rt, es_ffn, es_cb = (ExitStack(), ExitStack(),
                                        ExitStack(), ExitStack())
        def mk(es, name, bufs, space="SBUF"):
            return es.enter_context(tc.tile_pool(name=name, bufs=bufs,
                                                 space=space))

        # global pools (live for whole kernel)
        ps = mk(es_glob, "ps", 8, "PSUM")
        const = mk(es_glob, "const", 1)
        dram = mk(es_glob, "dram", 1, "DRAM")
        p_ct = mk(es_glob, "p_ct", 1)
        pools = {"ps": ps}

        # ---------- constants ----------
        ones128 = const.tile([128, 128], F32, tag="ones128", name="ones128")
        nc.vector.memset(ones128[:], 1.0)
        ident = const.tile([128, 128], F32, tag="ident", name="ident")
        nc.gpsimd.affine_select(ident[:], ones128[:], pattern=[[1, 128]],
                                base=0, channel_multiplier=-1,
                                compare_op=ALU.is_equal, fill=0.0)
        utri = const.tile([128, 128], F32, tag="utri", name="utri")
        nc.gpsimd.affine_select(utri[:], ones128[:], pattern=[[1, 128]],
                                base=0, channel_multiplier=-1,
                                compare_op=ALU.is_ge, fill=0.0)
        # iota over capacity slots: value c+1 at slot c (c<31), -1 at c=31
        iota_f = const.tile([128, E * CAP], F32, tag="iota_f", name="iota_f")
        nc.gpsimd.iota(iota_f[:], pattern=[[0, E], [1, CAP]], base=1,
                       channel_multiplier=0,
                       allow_small_or_imprecise_dtypes=True)
        iota_3d = iota_f[:].rearrange("p (e c) -> p e c", e=E)
        nc.vector.memset(iota_3d[:, :, CAP - 1:CAP], -1.0)
        gw_sb = const.tile([128, DC * E], F32, tag="gw", name="gw")
        for dc in range(DC):
            nc.sync.dma_start(gw_sb[:, dc * E:(dc + 1) * E],
                              gw_d[dc * 128:(dc + 1) * 128, :])
        gb_sb = const.tile([1, E], F32, tag="gb", name="gb")
        nc.sync.dma_start(gb_sb[:], gb_d[:])
        ones1 = const.tile([1, 128], F32, tag="ones1", name="ones1")
        nc.vector.memset(ones1[:], 1.0)

        # ---------- persistent tensors ----------
        p_xtb = mk(es_xtb, "p_xtb", 1)
        p_dT = mk(es_dT, "p_dT", 1)
        xTb = [p_xtb.tile([128, TOK], BF16, tag=f"xtb{dc}", name=f"xtb{dc}")
               for dc in range(DC)]
        combT = [p_ct.tile([128, NG * 128], BF16, tag=f"ct{ch}",
                           name=f"ct{ch}") for ch in range(2)]
        dispT = [p_dT.tile([128, NG * E * CAP], BF16, tag=f"dT{dc}",
                           name=f"dT{dc}") for dc in range(DC)]

        # DRAM scratch
        eo_dram = [dram.tile([NG * 128, D], BF16, tag=f"eo_dram{h}",
                             name=f"eo_dram{h}") for h in range(2)]
        sh_dram = dram.tile([TOK, D], BF16, tag="sh_dram", name="sh_dram")
        shk_b16 = [dram.tile([128, H], BF16, tag=f"shk_b16_{dc}",
                             name=f"shk_b16_{dc}") for dc in range(DC)]
        shv_b16 = [dram.tile([512, D], BF16, tag=f"shv_b16_{i}",
                             name=f"shv_b16_{i}") for i in range(DC)]

        # ---------- pre-cast shared weights to bf16 DRAM ----------
        # on the gpsimd (SWDGE) queue so it doesn't block the router's
        # sync-queue DMAs; overlaps with router compute
        p_pcf = mk(es_pre, "p_pcf", 2)
        p_pcb = mk(es_pre, "p_pcb", 2)
        for dc in range(DC):
            for hf in range(2):
                pf = p_pcf.tile([128, H // 2], F32, tag="pcf", name="pcf")
                nc.gpsimd.dma_start(pf[:], sk_d[dc * 128:(dc + 1) * 128,
                                                hf * 2048:(hf + 1) * 2048])
                pb = p_pcb.tile([128, H // 2], BF16, tag="pcb", name="pcb")
                nc.vector.tensor_copy(pb[:], pf[:])
                nc.gpsimd.dma_start(shk_b16[dc][:, hf * 2048:(hf + 1) * 2048],
                                    pb[:])
        for hc in range(HC):
            pf = p_pcf.tile([128, D], F32, tag="pcf", name="pcf")
            nc.gpsimd.dma_start(pf[:], sv_d[hc * 128:(hc + 1) * 128, :])
            pb = p_pcb.tile([128, D], BF16, tag="pcb", name="pcb")
            nc.vector.tensor_copy(pb[:], pf[:])
            nc.gpsimd.dma_start(shv_b16[hc // 4][(hc % 4) * 128:
                                                  (hc % 4 + 1) * 128, :], pb[:])

        # ---------- router + dispatch (per group) ----------
        p_xg = mk(es_rt, "p_xg", 6)
        p_xgb = mk(es_rt, "p_xgb", 2)
        p_dm = mk(es_rt, "p_dm", 2)
        p_xtf = mk(es_rt, "p_xtf", 10)
        p_sm = mk(es_rt, "p_sm", 8)
        p_sm8 = mk(es_rt, "p_sm8", 8)
        p_cmp = mk(es_rt, "p_cmp", 3)
        for g in range(NG):
            xg = p_xg.tile([128, D], F32, tag="xg", name="xg")
            nc.sync.dma_start(xg[:], x_d[g * 128:(g + 1) * 128, :])
            xgb = p_xgb.tile([128, D], BF16, tag="xgb", name="xgb")
            nc.scalar.copy(xgb[:], xg[:])

            # transpose x group: pack 4 [128,128] transposes per PSUM bank
            xtf = []
            for dc4 in range(2):
                tp = ps.tile([128, 512], F32, tag="ps", name="ps")
                for j in range(4):
                    dc = dc4 * 4 + j
                    nc.tensor.transpose(tp[:, j * 128:(j + 1) * 128],
                                        xg[:, dc * 128:(dc + 1) * 128],
                                        ident[:])
                t = p_xtf.tile([128, 512], F32, tag="xtf", name="xtf")
                nc.vector.tensor_copy(t[:], tp[:])
                xtf.append(t)
                for j in range(4):
                    dc = dc4 * 4 + j
                    nc.scalar.copy(xTb[dc][:, g * 128:(g + 1) * 128],
                                   tp[:, j * 128:(j + 1) * 128])
            # logits (cols 0:8) + cumsum positions (cols 8:16, 16:24) share
            # one PSUM bank
            lp = ps.tile([128, 3 * E], F32, tag="ps", name="ps")
            logits = lp[:, 0:E]
            for dc in range(DC):
                nc.tensor.matmul(logits, xtf[dc // 4][:, (dc % 4) * 128:
                                                      (dc % 4 + 1) * 128],
                                 gw_sb[:, dc * E:(dc + 1) * E],
                                 start=(dc == 0), stop=False)
            nc.tensor.matmul(logits, ones1[:], gb_sb[:],
                             start=False, stop=True)
            negm = p_sm.tile([128, 1], F32, tag="negm", name="negm")
            nc.vector.tensor_reduce(negm[:], logits,
                                    axis=mybir.AxisListType.X,
                                    op=ALU.max, negate=True)
            ex = p_sm8.tile([128, E], F32, tag="ex", name="ex")
            den = p_sm.tile([128, 1], F32, tag="den", name="den")
            nc.scalar.activation(ex[:], logits, AF.Exp, bias=negm[:],
                                 scale=1.0, accum_out=den[:])
            rec = p_sm.tile([128, 1], F32, tag="rec", name="rec")
            nc.vector.reciprocal(rec[:], den[:])
            probs = p_sm8.tile([128, E], F32, tag="probs", name="probs")
            nc.vector.tensor_scalar_mul(probs[:], ex[:], rec[:])
            m1 = p_sm.tile([128, 1], F32, tag="m1", name="m1")
            nc.vector.reduce_max(m1[:], probs[:], axis=mybir.AxisListType.X)
            mask1 = p_sm8.tile([128, E], F32, tag="mask1", name="mask1")
            nc.vector.tensor_scalar(mask1[:], probs[:], m1[:], None,
                                    op0=ALU.is_ge)
            probs2 = p_sm8.tile([128, E], F32, tag="probs2", name="probs2")
            nc.vector.scalar_tensor_tensor(probs2[:], mask1[:], -1e30,
                                           probs[:], ALU.mult, ALU.add)
            m2 = p_sm.tile([128, 1], F32, tag="m2", name="m2")
            nc.vector.reduce_max(m2[:], probs2[:], axis=mybir.AxisListType.X)
            mask2 = p_sm8.tile([128, E], F32, tag="mask2", name="mask2")
            nc.vector.tensor_scalar(mask2[:], probs2[:], m2[:], None,
                                    op0=ALU.is_ge)
            # positions: inclusive cumsum over tokens (partition dim) via
            # upper-triangular matmul, then mask to assigned experts
            pos = []
            for ki, mask in enumerate((mask1, mask2)):
                pp = lp[:, (ki + 1) * E:(ki + 2) * E]
                nc.tensor.matmul(pp, utri[:], mask[:], start=True, stop=True)
                pm = p_sm8.tile([128, E], F32, tag="pos", name="pos")
                nc.vector.tensor_mul(pm[:], pp, mask[:])
                pos.append(pm)
            cmp1 = p_cmp.tile([128, E * CAP], F32, tag="cmp1", name="cmp1")
            nc.vector.tensor_tensor(
                cmp1[:].rearrange("p (e c) -> p e c", e=E),
                pos[0][:].unsqueeze(2).broadcast_to([128, E, CAP]),
                iota_3d, op=ALU.is_equal)
            cmp2 = p_cmp.tile([128, E * CAP], F32, tag="cmp2", name="cmp2")
            nc.vector.tensor_tensor(
                cmp2[:].rearrange("p (e c) -> p e c", e=E),
                pos[1][:].unsqueeze(2).broadcast_to([128, E, CAP]),
                iota_3d, op=ALU.is_equal)
            dm = p_dm.tile([128, E * CAP], BF16, tag="dm", name="dm")
            nc.vector.tensor_add(dm[:], cmp1[:], cmp2[:])
            cmp2s = p_cmp.tile([128, E * CAP], F32, tag="cmp2s", name="cmp2s")
            nc.vector.tensor_scalar_mul(cmp2s[:], cmp2[:], m2[:])
            comb = p_cmp.tile([128, E * CAP], F32, tag="comb", name="comb")
            nc.vector.scalar_tensor_tensor(comb[:], cmp1[:], m1[:],
                                           cmp2s[:], ALU.mult, ALU.add)
            ctp = ps.tile([128, 256], F32, tag="ps", name="ps")
            for ch in range(2):
                nc.tensor.transpose(ctp[:, ch * 128:(ch + 1) * 128],
                                    comb[:, ch * 128:(ch + 1) * 128],
                                    ident[:])
            for ch in range(2):
                nc.vector.tensor_copy(combT[ch][:, g * 128:(g + 1) * 128],
                                      ctp[:, ch * 128:(ch + 1) * 128])
            # dispatch matmul for this group (fills PE gaps in the
            # router's serial DVE chain)
            for dcp in range(4):
                    dps = ps.tile([128, 512], F32, tag="ps", name="ps")
                    for j in range(2):
                        dc = dcp * 2 + j
                        nc.tensor.matmul(dps[:, j * 256:(j + 1) * 256],
                                         xgb[:, dc * 128:(dc + 1) * 128],
                                         dm[:], start=True, stop=True)
                    for j in range(2):
                        dc = dcp * 2 + j
                        dst = dispT[dc][:, g * E * CAP:(g + 1) * E * CAP]
                        if j == 0:
                            nc.vector.tensor_copy(dst, dps[:, 0:256])
                        else:
                            nc.scalar.copy(dst, dps[:, 256:512])
        es_rt.close()
        es_pre.close()

        # ---------- FFN units ----------
        pools["kf"] = mk(es_ffn, "p_kf", 8)
        pools["kb"] = mk(es_ffn, "p_kb", 8)
        pools["vf"] = mk(es_ffn, "p_vf", 6)
        pools["vb"] = mk(es_ffn, "p_vb", 6)
        pools["hid"] = mk(es_ffn, "p_hid", 1)
        pools["eo"] = mk(es_ffn, "p_eo", 3)

        # shared expert: 4 token-quarters
        for q in range(4):
            _emit_ffn_unit(
                nc, pools,
                rhs_fn=lambda dc, q=q: xTb[dc][:, q * 512:(q + 1) * 512],
                out_ap_fn=lambda sc, q=q: sh_dram[q * 512 + sc * 128:
                                                  q * 512 + (sc + 1) * 128, :],
                keys_b16=shk_b16, vals_b16=shv_b16)

        # routed experts
        eo_g = [t[:].rearrange("(g ec) d -> g ec d", ec=128)
                for t in eo_dram]
        for e in range(E):
            def rhs_fn(dc, e=e):
                r = dispT[dc][:].rearrange("p (g ec) -> p g ec", g=NG)
                return r[:, :, e * CAP:(e + 1) * CAP]
            def out_ap_fn(sc, e=e):
                # FFN2 tile rows are slots (g-major): row r -> group
                # sc*4 + r//32, capacity slot r%32 of expert e
                return eo_g[e // 4][sc * 4:(sc + 1) * 4,
                                    (e % 4) * CAP:(e % 4 + 1) * CAP, :]
            _emit_ffn_unit(
                nc, pools, rhs_fn=rhs_fn, out_ap_fn=out_ap_fn,
                keys_f32=k_d[e], vals_f32=v_d[e])
        es_ffn.close()
        es_dT.close()

        # ---------- combine ----------
        p_eg = mk(es_cb, "p_eg", 8)
        p_shg = mk(es_cb, "p_shg", 4)
        p_ot = mk(es_cb, "p_ot", 4)
        for g in range(NG):
            eg = [p_eg.tile([128, D], BF16, tag="eg", name="eg")
                  for _ in range(2)]
            for ch in range(2):
                r0 = g * 128
                nc.sync.dma_start(eg[ch][:], eo_dram[ch][r0:r0 + 128, :])
            shg = p_shg.tile([128, D], BF16, tag="shg", name="shg")
            nc.sync.dma_start(shg[:], sh_dram[g * 128:(g + 1) * 128, :])
            psA = ps.tile([128, 512], F32, tag="ps", name="ps")
            psB = ps.tile([128, 512], F32, tag="ps", name="ps")
            for ch in range(2):
                lhsT = combT[ch][:, g * 128:(g + 1) * 128]
                nc.tensor.matmul(psA[:], lhsT, eg[ch][:, 0:512],
                                 start=(ch == 0), stop=(ch == 1))
                nc.tensor.matmul(psB[:], lhsT, eg[ch][:, 512:1024],
                                 start=(ch == 0), stop=(ch == 1))
            ot = p_ot.tile([128, D], F32, tag="ot", name="ot")
            nc.vector.tensor_tensor(ot[:, 0:512], psA[:], shg[:, 0:512],
                                    op=ALU.add)
            nc.vector.tensor_tensor(ot[:, 512:1024], psB[:],
                                    shg[:, 512:1024], op=ALU.add)
            nc.gpsimd.dma_start(out_d[g * 128:(g + 1) * 128, :], ot[:])
        es_cb.close()
        es_xtb.close()

    nc.compile()
    return nc


LAST_RESULT = None


def kernel(x, gate_weight, gate_bias, keys, values, shared_keys,
           shared_values, **_ignored):
    global LAST_RESULT
    _ensure_ntff_hook()
    x = np.ascontiguousarray(np.asarray(x, dtype=np.float32))
    gate_weight = np.ascontiguousarray(np.asarray(gate_weight, np.float32))
    gate_bias = np.ascontiguousarray(
        np.asarray(gate_bias, np.float32).reshape(1, E))
    keys = np.ascontiguousarray(np.asarray(keys, np.float32))
    values = np.ascontiguousarray(np.asarray(values, np.float32))
    shared_keys = np.ascontiguousarray(
        np.asarray(shared_keys, np.float32).reshape(D, H))
    shared_values = np.ascontiguousarray(
        np.asarray(shared_values, np.float32).reshape(H, D))

    if "nc" not in _CACHE:
        _CACHE["nc"] = _build_program()
    nc = _CACHE["nc"]

    xt = x.reshape(NCORES, TOK, D)
    in_maps = []
    for i in range(NCORES):
        in_maps.append({
            "x_s": np.ascontiguousarray(xt[i]),
            "gw": gate_weight,
            "gb": gate_bias,
            "keys": keys,
            "values": values,
            "shk": shared_keys,
            "shv": shared_values,
        })
    res = run_bass_kernel_spmd(nc, in_maps, core_ids=list(range(NCORES)))
    LAST_RESULT = res
    out = np.concatenate([res.results[i]["out"] for i in range(NCORES)],
                         axis=0)
    return out.reshape(B, S, D).astype(np.float32)

